# revision 59
# baseline (speedup 1.0000x reference)
"""Trainium2 Bass kernel for nn_DisentangledSelfAttention.

Sharding: batch (B=8) across the 8 NeuronCores, weights replicated.

Host-side algebra (exact identities, done in numpy inside kernel()):
  q = relu(x @ (W_Q @ Wq_w.T) + bq)   -- no nonlinearity between the two
  k = relu(x @ (W_K @ Wk_w.T) + bk)      projection stages, so they fold
  v = relu(x @ (W_V @ Wv_w.T) + bv)      into one [E, A] matrix each
  uw = softmax_l(x @ (W_K @ Wu_w.T) + bu)   (tiny: E*H mults)

fp8 strategy (all scales powers of two, folded into constants):
  xT8 = fp8(x.T * 16), W8 = fp8(W_eff * 64)  -> projections run as
  fp8e4m3 DoubleRow matmuls (4x fewer PE cycles); biases pre-scaled by
  1024 so relu commutes (relu(a*z) = a*relu(z) for a>0).  q/k stay bf16
  (= 1024x true) feeding the bf16 pair matmul; exp scale absorbs 1/1024^2.
  v8 = fp8(8*v_true) via a *2^-7 on the PSUM; PT = fp8(exp(logit - C))
  with C=1 a global logit shift (cancels in the softmax ratio, keeps PT
  inside e4m3 range).  PV + den matmuls are fp8 DoubleRow over t-slot
  pairs; den's ones are memset to 8.0 so 1/ps_den also cancels v8's 8x.

Device math per core (one batch item; L=1024, A=512, H=8, HD=64):
  qT/kT = relu(W.T @ xT + b)  [A, L]   (lhsT = W_eff natural, rhs = xT)
  v     = relu(x @ Wv + b)    [L, A]   (lhsT = xT, rhs = Wv_eff)
  The torch .view group reshape makes attention block-diagonal over
  128-row l-blocks (group g), with pseudo-seq s -> (l=128g+r, a=64c+d).
  We enumerate s as (par, ac, r) with c = 2*ac+par so that qT partition
  halves are directly the pair-matmul rhs.  kdup duplicates kT group
  slices into both partition halves (slot t = head-col c'=t) so the
  pair lhsT is available at either base partition; free-dim reduction
  on kdup gives the true group mean (both halves hold all of c').
  k-centering cancels in the softmax (it only shifts logits by a
  per-query constant), so k is used raw; q centering folds into the
  exp bias:  S = q . k - (sum_s q/1024) . k[s']   (per-s'-row bias)
  PT[t] = exp(S*scale + ebias) (fp8), then out[s-block, d] accumulates
  DoubleRow PT[t-pair].T @ v8[t-pair] (N=64 moving dim), denominators
  via rhs=eights K=2x128 DR matmuls, unary rank-1 term via host uw:
     out = psum * (1/den8) + uwv   in one DVE scalar_tensor_tensor.
"""

import os
import numpy as np

B, L, E, A, H, HD = 8, 1024, 1024, 512, 8, 64
G = 8
N_CORES = 8

S_X = 16.0          # fp8 scale on x
S_W = 64.0          # fp8 scale on folded weights
S_QK = S_X * S_W    # scale of on-chip q and k (1024)
S_U = 128.0         # fp8 scale on unary softmax weights
LOGC = 1.0          # global logit shift (cancels in softmax ratio)
EXPS = 1.0 / (8.0 * S_QK * S_QK)            # exp scale on raw q.k PSUM
EBMUL = -1.0 / (8.0 * 1024.0 * S_QK * S_QK)  # ebias mult on (sum_q).k PSUM
# fp8 bit-trick exp (DVE-offloaded slots): e4m3 bits = round(ALPHA*logit
# + 56 + SIGMA); uint8 convert is RNE on HW, sigma fitted for zero mean
# relative error (~3.1% rms, cancels in the softmax ratio).
EXP8A = 8.0 / np.log(2.0)
EXP8B = 56.0 - 0.45
DVE_EXP_SLOTS = (4, 5)


def _build_nc():
    from contextlib import ExitStack

    import concourse.bass as bass
    import concourse.tile as tile
    import concourse.mybir as mybir
    from concourse import bacc

    f32 = mybir.dt.float32
    bf16 = mybir.dt.bfloat16
    f8 = mybir.dt.float8e4
    u8 = mybir.dt.uint8
    DR = mybir.MatmulPerfMode.DoubleRow
    Alu = mybir.AluOpType
    Act = mybir.ActivationFunctionType

    nc = bacc.Bacc("TRN2", target_bir_lowering=False, debug=False,
                   num_devices=N_CORES)

    xT_d = nc.dram_tensor("xT", [E, L], f8, kind="ExternalInput").ap()
    Wq_d = nc.dram_tensor("Wq", [128, 8, A], f8, kind="ExternalInput").ap()
    Wk_d = nc.dram_tensor("Wk", [128, 8, A], f8, kind="ExternalInput").ap()
    Wv_d = nc.dram_tensor("Wv", [128, 8, A], f8, kind="ExternalInput").ap()
    bq_d = nc.dram_tensor("bq", [128, 4], f32, kind="ExternalInput").ap()
    bk_d = nc.dram_tensor("bk", [128, 4], f32, kind="ExternalInput").ap()
    bv_d = nc.dram_tensor("bv", [1, A], bf16, kind="ExternalInput").ap()
    uc_d = nc.dram_tensor("ucol", [128, G, H], f8, kind="ExternalInput").ap()
    out_d = nc.dram_tensor("out", [L, A], f32, kind="ExternalOutput").ap()

    with tile.TileContext(nc) as tc, ExitStack() as ctx:
        persist = ctx.enter_context(tc.tile_pool(name="persist", bufs=1))
        pt_pool = ctx.enter_context(tc.tile_pool(name="pt", bufs=2))
        ost_pool = ctx.enter_context(tc.tile_pool(name="ost", bufs=2))
        small = ctx.enter_context(tc.tile_pool(name="small", bufs=24))
        # PSUM budget (8 banks): p_pair 4 (S+exp double-buffer), p_o 2 (PV
        # accumulators), p_misc 1 (prep/den/uwv regions, both parities),
        # p_fill 1 (projection filler chains, two [128,256] regions)
        p_pair = ctx.enter_context(tc.tile_pool(name="p_pair", bufs=2, space="PSUM"))
        p_o = ctx.enter_context(tc.tile_pool(name="p_o", bufs=2, space="PSUM"))
        p_misc = ctx.enter_context(tc.tile_pool(name="p_misc", bufs=1, space="PSUM"))
        p_fill = ctx.enter_context(tc.tile_pool(name="p_fill", bufs=1, space="PSUM"))

        # persistent manually-regioned PSUM tiles (1 bank each)
        ps_misc = p_misc.tile([128, 160], f32, tag="misc")
        ps_fillbuf = p_fill.tile([128, 512], f32, tag="fillbuf")
        fill_ctr = [0]

        def fill_region():
            i = fill_ctr[0] % 2
            fill_ctr[0] += 1
            return ps_fillbuf[:, 256 * i:256 * i + 256]

        ones_row = persist.tile([1, 128], bf16, tag="ones_row")
        nc.vector.memset(ones_row, 1.0)
        eights2 = persist.tile([128, 2, 1], f8, tag="eights2")
        nc.vector.memset(eights2, 8.0)
        zeros = persist.tile([128, 256], bf16, tag="zeros")
        nc.vector.memset(zeros, 0.0)

        xT = persist.tile([128, 8, L], f8, tag="xT")
        wk_sb = persist.tile([128, 8, A], f8, tag="wk_sb")
        wq_sb = persist.tile([128, 8, A], f8, tag="wq_sb")
        wv_sb = persist.tile([128, 8, A], f8, tag="wv_sb")
        x_src = xT_d.rearrange("(ec p) l -> p ec l", p=128)
        # first k-chain inputs stream on the HWDGE queue in consumption
        # order; everything not needed immediately goes via the Pool/SWDGE
        # queue so the two generators run in parallel.
        bk_sb = persist.tile([128, 4], f32, tag="bk_sb")
        nc.gpsimd.dma_start(bk_sb, bk_d)
        for h4 in range(2):
            nc.sync.dma_start(wk_sb[:, 4 * h4:4 * h4 + 4, :],
                              Wk_d[:, 4 * h4:4 * h4 + 4, :])
            nc.sync.dma_start(xT[:, 4 * h4:4 * h4 + 4, 0:256],
                              x_src[:, 4 * h4:4 * h4 + 4, 0:256])
        bq_sb = persist.tile([128, 4], f32, tag="bq_sb")
        nc.gpsimd.dma_start(bq_sb, bq_d)
        nc.sync.dma_start(wq_sb[:, 0:4, :], Wq_d[:, 0:4, :])
        nc.sync.dma_start(wq_sb[:, 4:8, :], Wq_d[:, 4:8, :])
        nc.sync.dma_start(xT[:, :, 256:512], x_src[:, :, 256:512])
        nc.sync.dma_start(wv_sb[:, 0:4, :], Wv_d[:, 0:4, :])
        nc.sync.dma_start(wv_sb[:, 4:8, :], Wv_d[:, 4:8, :])
        bv_sb = persist.tile([1, A], bf16, tag="bv_sb")
        nc.gpsimd.dma_start(bv_sb, bv_d)
        uc_sb = persist.tile([128, G, H, 1], f8, tag="uc_sb")
        nc.gpsimd.dma_start(uc_sb.rearrange("p g h one -> p g (h one)"), uc_d)
        # (lq2/lq3 windows are prefetched inside the group loop)

        qT = persist.tile([128, 4, L], bf16, tag="qT")
        kT = persist.tile([128, 4, L], bf16, tag="kT")
        # parity-crossed k copies only: upper half holds even t-slots
        # (native in kT's lower half), lower half holds odd t-slots.
        kdup = persist.tile([128, G, 4, 128], bf16, tag="kdup")
        v_all = persist.tile([128, 8, A], f8, tag="v_all")
        qpartb = persist.tile([128, G], bf16, tag="qpartb")
        # total q-sum (even+odd parity) duplicated into both halves
        qsum_tot = persist.tile([128, G], bf16, tag="qsum_tot")
        ebias = persist.tile([128, G, 8], f32, tag="ebias")
        # bit-trick bias for DVE-offloaded exp slots
        ebias8 = persist.tile([128, G, 8], f32, tag="ebias8")

        # v views per group: [p, t-pair, t-in-pair, d]
        v_g = v_all.rearrange("p g (tp two d) -> p g tp two d", two=2, d=64)

        def k_half(t, g, lo):
            # k[d, t-slot, r'] for partitions [0:64) if lo else [64:128)
            tt = t // 2
            if (t % 2 == 0) == lo:       # native parity lives in kT
                base = slice(0, 64) if lo else slice(64, 128)
                return kT[base, tt, 128 * g:128 * g + 128]
            base = slice(0, 64) if lo else slice(64, 128)
            return kdup[base, g, tt, :]

        def qk_chain(w_sb, b_sb, dst, lq, ac, accs=None, ps=None):
            # dst[:, ac, 256lq:+256] = relu(W.T @ xT + b), fp8 DoubleRow
            if ps is None:
                ps = fill_region()
            for e2 in range(4):
                nc.tensor.matmul(
                    ps, w_sb[:, 2 * e2:2 * e2 + 2,
                             128 * ac:128 * ac + 128],
                    xT[:, 2 * e2:2 * e2 + 2, 256 * lq:256 * lq + 256],
                    start=(e2 == 0), stop=(e2 == 3), perf_mode=DR)
            if accs is None:
                # k path needs no per-group accumulators: one wide relu
                nc.vector.scalar_tensor_tensor(
                    out=dst[:, ac, 256 * lq:256 * lq + 256], in0=ps,
                    scalar=b_sb[:, ac:ac + 1], in1=zeros[:, 0:256],
                    op0=Alu.add, op1=Alu.max)
                return
            for j in range(2):
                nc.vector.scalar_tensor_tensor(
                    out=dst[:, ac, 256 * lq + 128 * j:256 * lq + 128 * j + 128],
                    in0=ps[:, 128 * j:128 * j + 128],
                    scalar=b_sb[:, ac:ac + 1], in1=zeros[:, 0:128],
                    op0=Alu.add, op1=Alu.max,
                    accum_out=accs[j][:, ac:ac + 1])

        def v_proj(lt, half):
            # v_all[:, lt, 256*half:+256] = fp8(2^-7 * (x @ Wv + bv))
            a0 = 256 * half
            ps = fill_region()
            for e2 in range(4):
                nc.tensor.matmul(
                    ps, xT[:, 2 * e2:2 * e2 + 2, 128 * lt:128 * lt + 128],
                    wv_sb[:, 2 * e2:2 * e2 + 2, a0:a0 + 256],
                    start=(e2 == 0), stop=False, perf_mode=DR)
            nc.tensor.matmul(ps, ones_row, bv_sb[:, a0:a0 + 256],
                             start=False, stop=True)
            nc.vector.tensor_scalar(out=v_all[:, lt, a0:a0 + 256], in0=ps,
                                    scalar1=2.0 ** -7, scalar2=0.0,
                                    op0=Alu.mult, op1=Alu.max)

        def kdup_fill_g(g, eng):
            # copy only parity-crossed halves: kT lower (even slots) -> kdup
            # upper, kT upper (odd slots) -> kdup lower
            i = g % 2
            sl = slice(256 * (g // 2), 256 * (g // 2) + 256)
            src_lo = kT[0:64, :, sl].rearrange("p ac (g r) -> p g ac r", r=128)
            src_hi = kT[64:128, :, sl].rearrange("p ac (g r) -> p g ac r", r=128)
            eng.dma_start(kdup[64:128, g, :, :], src_lo[:, i])
            eng.dma_start(kdup[0:64, g, :, :], src_hi[:, i])

        def kdup_fill(lq):
            kdup_fill_g(2 * lq, nc.gpsimd)
            kdup_fill_g(2 * lq + 1, nc.gpsimd)

        def prep_group(g, qaccs):
            # q group sums (both halves), then ebias[s'] = logit bias - C
            qa = qaccs[g % 2]
            t1 = small.tile([128, 1], f32, tag="t1", name=f"t1_{g}")
            t2 = small.tile([128, 1], f32, tag="t2", name=f"t2_{g}")
            nc.gpsimd.tensor_add(t1, qa[:, 0:1], qa[:, 1:2])
            nc.gpsimd.tensor_add(t2, qa[:, 2:3], qa[:, 3:4])
            nc.gpsimd.tensor_add(qpartb[:, g:g + 1], t1, t2)
            qtmp = small.tile([64, 1], bf16, tag="qtmp", name=f"qtmp_{g}")
            nc.vector.tensor_copy(out=qtmp, in_=qpartb[64:128, g:g + 1])
            nc.gpsimd.tensor_add(qsum_tot[0:64, g:g + 1],
                                 qpartb[0:64, g:g + 1], qtmp)
            nc.vector.tensor_copy(out=qsum_tot[64:128, g:g + 1],
                                  in_=qsum_tot[0:64, g:g + 1])
            ps_b = ps_misc[:, 80 * (g % 2):80 * (g % 2) + 8]
            for t in range(8):
                # native kT half only (no kdup dependency): contract with
                # the duplicated total q-sum at the matching base partition
                par = t % 2
                nc.tensor.matmul(ps_b[:, t:t + 1],
                                 kT[64 * par:64 * par + 64, t // 2,
                                    128 * g:128 * g + 128],
                                 qsum_tot[64 * par:64 * par + 64, g:g + 1],
                                 start=(t == 0), stop=(t == 7))
            nc.vector.tensor_scalar(out=ebias[:, g, :], in0=ps_b,
                                    scalar1=EBMUL, scalar2=-LOGC,
                                    op0=Alu.mult, op1=Alu.add)
            nc.gpsimd.tensor_scalar(out=ebias8[:, g, :], in0=ebias[:, g, :],
                                    scalar1=EXP8A, scalar2=EXP8B,
                                    op0=Alu.mult, op1=Alu.add)

        def main_group(g, fillers=(), pre_work=None, pre_dve=None,
                       misc_work=()):
            fillers = list(fillers)
            misc_work = list(misc_work)
            PT = pt_pool.tile([128, 8, 1024], f8, tag="PT", name=f"PT_{g}")
            z = 80 * (g % 2)
            ps_den = ps_misc[:, z + 8:z + 16]
            ps_ot = p_o.tile([128, 8, 64], f32, tag="ps_o", name=f"o_{g}")
            ps_os = [ps_ot[:, h, :] for h in range(8)]

            def emit_uwv():
                # unary rank-1 term: uwv = sum_t uw[:, t] . v[:, 64t:+64]
                ps_uwv = ps_misc[0:1, z + 16:z + 80]
                for t in range(8):
                    nc.tensor.matmul(ps_uwv, uc_sb[:, g, t, :],
                                     v_all[:, g, 64 * t:64 * t + 64],
                                     start=(t == 0), stop=(t == 7))
                uwv_sb = small.tile([1, 64], bf16, tag="uwv_sb",
                                    name=f"uwvs_{g}")
                nc.vector.tensor_scalar_mul(uwv_sb, ps_uwv, 2.0 ** -10)
                ubc_sb = small.tile([128, 64], bf16, tag="ubc_sb",
                                    name=f"ubcs_{g}")
                nc.gpsimd.partition_broadcast(ubc_sb, uwv_sb)
                return ubc_sb

            def emit_pvden(tp):
                for h in range(8):
                    nc.tensor.matmul(
                        ps_os[h], PT[:, 2 * tp:2 * tp + 2, 128 * h:128 * h + 128],
                        v_g[:, g, tp, :, :],
                        start=(tp == 0 and h == 0),
                        stop=(tp == 3 and h == 7), perf_mode=DR)
                for h in range(8):
                    nc.tensor.matmul(
                        ps_den[:, h:h + 1],
                        PT[:, 2 * tp:2 * tp + 2, 128 * h:128 * h + 128],
                        eights2, start=(tp == 0 and h == 0),
                        stop=(tp == 3 and h == 7), perf_mode=DR)

            ubc_sb = None
            for t in range(8):
                ps_S = p_pair.tile([128, 1024], f32, tag="pair",
                                   name=f"S_{g}_{t}")
                nc.tensor.matmul(ps_S[:, 0:512], k_half(t, g, True),
                                 qT[0:64, :, 128 * g:128 * g + 128],
                                 start=True, stop=True)
                nc.tensor.matmul(ps_S[:, 512:1024], k_half(t, g, False),
                                 qT[64:128, :, 128 * g:128 * g + 128],
                                 start=True, stop=True)
                if t in DVE_EXP_SLOTS:
                    # fp8 bit-trick exp on DVE to offload the ACT engine
                    nc.vector.tensor_scalar(
                        out=PT[:, t, :].bitcast(u8), in0=ps_S,
                        scalar1=EXP8A * EXPS,
                        scalar2=ebias8[:, g, t:t + 1],
                        op0=Alu.mult, op1=Alu.add)
                else:
                    nc.scalar.activation(
                        out=PT[:, t, :], in_=ps_S, func=Act.Exp,
                        bias=ebias[:, g, t:t + 1], scale=EXPS)
                # Misc-bank (prep/den/uwv) discipline: den(prev) is closed by
                # pre_work at t==1; all other misc-bank matmuls (uwv, preps)
                # go at t==2, before den(g) opens at t==3.  This keeps at
                # most one accumulation group open per PSUM bank at any time
                # (an intervening group corrupts an open one on HW).
                if t == 1 and pre_work is not None:
                    pre_work()
                if t == 1 and pre_dve is not None:
                    pre_dve()
                if t == 2:
                    ubc_sb = emit_uwv()
                    for w in misc_work:
                        w()
                elif t in (0, 1, 6, 7):
                    # filler slots chosen so their DVE relu/convert work
                    # never queues ahead of the DVE-offloaded exp slots
                    for _ in range(1 if t < 2 else 3):
                        if fillers:
                            fillers.pop(0)()
                # PV/den for pair j emitted after S(2j+3), so when the
                # in-order PE sequencer reaches them exp(2j+1) has already
                # completed (S(2j+3) waited on it via p_pair rotation).
                if t >= 3 and t % 2 == 1 and t < 7:
                    emit_pvden((t - 3) // 2)
                if t == 7:
                    emit_pvden(2)

            while fillers:
                fillers.pop(0)()

            def flush_pe():
                # last PV/den pair; called from inside the next group after
                # its S(1) (closes this group's den accumulation group).
                emit_pvden(3)

            def flush_dve():
                # normalization tail; deferred to the next group's t==4 so
                # the DVE-offloaded exp slots are not queued behind it.
                rcol = small.tile([128, 8], f32, tag="rcol", name=f"rcol_{g}")
                nc.vector.reciprocal(out=rcol, in_=ps_den)
                ostage = ost_pool.tile([128, A], f32, tag="ostage",
                                       name=f"ost_{g}")
                for c in range(8):
                    h = (c % 2) * 4 + c // 2
                    nc.vector.scalar_tensor_tensor(
                        out=ostage[:, 64 * c:64 * c + 64], in0=ps_os[h],
                        scalar=rcol[:, h:h + 1], in1=ubc_sb,
                        op0=Alu.mult, op1=Alu.add)
                    if c == 3:
                        nc.sync.dma_start(out_d[128 * g:128 * g + 128, 0:256],
                                          ostage[:, 0:256])
                nc.sync.dma_start(out_d[128 * g:128 * g + 128, 256:512],
                                  ostage[:, 256:512])
            return flush_pe, flush_dve

        qaccs_by_lq = {}

        def mk_qaccs(lq):
            qaccs_by_lq[lq] = [
                small.tile([128, 4], f32, tag="qacc", name=f"qacc_{2*lq+j}")
                for j in range(2)]

        # prologue: minimal work for group 0 only.  Chains run in wide
        # p_pair tiles (4 regions each) so they are not serialized on the
        # two-region fill buffer; p_pair rotation resumes cleanly after.
        # preload the Exp activation table off the critical path
        warm = small.tile([1, 1], f32, tag="warm")
        nc.scalar.activation(out=warm, in_=ones_row[0:1, 0:1], func=Act.Exp)

        mk_qaccs(0)
        for ac in range(4):
            qk_chain(wk_sb, bk_sb, kT, 0, ac)
        kdup_fill_g(0, nc.sync)
        for ac in range(4):
            qk_chain(wq_sb, bq_sb, qT, 0, ac, accs=qaccs_by_lq[0])
        v_proj(0, 0)
        v_proj(0, 1)
        prep_group(0, qaccs_by_lq[0])
        kdup_fill_g(1, nc.gpsimd)

        def mk_preps(lq):
            prep_group(2 * lq, qaccs_by_lq[lq])
            prep_group(2 * lq + 1, qaccs_by_lq[lq])

        pending = None
        for lq in range(4):
            if lq < 2:
                w0 = 512 + 256 * lq
                nc.sync.dma_start(xT[:, :, w0:w0 + 256], x_src[:, :, w0:w0 + 256])
            if lq < 3:
                nlq = lq + 1
                mk_qaccs(nlq)
                kf = [(lambda ac=ac: qk_chain(wk_sb, bk_sb, kT, nlq, ac))
                      for ac in range(4)]
                qf = [(lambda ac=ac: qk_chain(wq_sb, bq_sb, qT, nlq, ac,
                                              accs=qaccs_by_lq[nlq]))
                      for ac in range(4)]
                vf = [(lambda lt=lt, h=h: v_proj(lt, h))
                      for lt in (2 * nlq, 2 * nlq + 1) for h in range(2)]
                fill0 = qf + kf
                fill1 = vf
                misc1 = [lambda: mk_preps(nlq)]
            else:
                fill0, fill1, misc1 = [], [], []
            if lq == 0:
                # group 0 also absorbs the work deferred from the prologue
                fill0 = [lambda h=h: v_proj(1, h) for h in range(2)] + fill0
                misc0 = [lambda: prep_group(1, qaccs_by_lq[0])]
            else:
                misc0 = []

            pending = main_group(2 * lq, fill0,
                                 pre_work=pending[0] if pending else None,
                                 pre_dve=pending[1] if pending else None,
                                 misc_work=misc0)
            if lq < 3:
                kdup_fill(lq + 1)
            pending = main_group(2 * lq + 1, fill1,
                                 pre_work=pending[0], pre_dve=pending[1],
                                 misc_work=misc1)
        pending[0]()
        pending[1]()

    nc.compile()
    return nc


def _host_prep(inputs):
    import ml_dtypes
    f8 = ml_dtypes.float8_e4m3
    f32 = np.float32
    g = {k: np.asarray(v, dtype=f32) for k, v in inputs.items()}
    Wq_eff = g["W_Q"] @ g["Wq_w"].T          # [E, A]
    Wk_eff = g["W_K"] @ g["Wk_w"].T
    Wv_eff = g["W_V"] @ g["Wv_w"].T
    Wu_eff = g["W_K"] @ g["Wu_w"].T          # [E, H]

    def chunk_w(w):  # [E, A] -> [128, 8, A] with [p, ec, a] = w[128*ec+p, a]
        return np.ascontiguousarray(
            (w * S_W).reshape(8, 128, A).transpose(1, 0, 2)).astype(f8)

    wq, wk, wv = chunk_w(Wq_eff), chunk_w(Wk_eff), chunk_w(Wv_eff)
    bq = np.ascontiguousarray(g["Wq_b"].reshape(4, 128).T) * S_QK
    bk = np.ascontiguousarray(g["Wk_b"].reshape(4, 128).T) * S_QK
    bv = (g["Wv_b"].reshape(1, A) * S_QK).astype(ml_dtypes.bfloat16)

    x = g["x"]                                # [B, L, E]
    unary = np.einsum("ble,eh->blh", x, Wu_eff) + g["Wu_b"]
    unary -= unary.max(axis=1, keepdims=True)
    eu = np.exp(unary)
    uw = eu / eu.sum(axis=1, keepdims=True)   # [B, L, H]

    per_core = []
    for b in range(B):
        xT = np.ascontiguousarray(x[b].T * S_X).astype(f8)
        ucol = np.ascontiguousarray(
            (uw[b] * S_U).reshape(G, 128, H).transpose(1, 0, 2)).astype(f8)
        per_core.append(dict(xT=xT, Wq=wq, Wk=wk, Wv=wv, bq=bq, bk=bk,
                             bv=bv, ucol=ucol))
    return per_core


_NC_CACHE = {}


def kernel(**inputs):
    from concourse.bass_utils import run_bass_kernel_spmd

    if "nc" not in _NC_CACHE:
        _NC_CACHE["nc"] = _build_nc()
    nc = _NC_CACHE["nc"]

    in_maps = _host_prep(inputs)

    trace = os.environ.get("KERNEL_TRACE", "0") == "1"
    # First execution after a fresh NEFF load occasionally hits a transient
    # NRT_EXEC_UNIT_UNRECOVERABLE; a retry on the reloaded device succeeds.
    last_exc = None
    for _attempt in range(3):
        try:
            res = run_bass_kernel_spmd(nc, in_maps,
                                       core_ids=list(range(N_CORES)),
                                       trace=trace)
            break
        except Exception as e:
            last_exc = e
    else:
        raise last_exc
    if trace and res.exec_time_ns is not None:
        print(f"HW exec time: {res.exec_time_ns} ns")
        kernel.last_exec_time_ns = res.exec_time_ns
    out = np.stack([r["out"] for r in res.results], axis=0)
    return out
